# revision 1
# baseline (speedup 1.0000x reference)
"""CrossPixContrastive loss on 8 trn2 NeuronCores.

Math (per batch n, HW=4096, C=256):
  rgb_n = l2norm_C(rgb); ir_n = l2norm_C(ir)
  e[p,q] = exp(20 * clip(<rgb_n[:,p], ir_n[:,q]>, -1, 1))
  S[p] = sum_q e ; M[p] = sum_q e * (rm_p == im_q)
  C[q] = sum_p e ; Mc[q] = sum_p e * (rm_p == im_q)
  r_rgb = M/(S+1e-6) ; r_ir = Mc/(C+1e-6)
  loss = mean(-log over nonzero of concat(r_rgb, r_ir) * fg)

Sharding: 8 cores = 4 batches x 2 halves of the rgb-pixel axis p.
Per-core tiling: [128p x 1024q] tiles of e.
  PE  : logit matmul (K=256, f32r inputs, fp32 accum) + column-sum matmuls
        with lhsT = [ones | onehot(rm) x5] (bf16) -> psum [6,512] over p
  ACT : e = Exp(logit) -> bf16, fp32 row-sum accum -> S
  DVE : (im==rm)*e fused scalar_tensor_tensor (bf16 2x) with accum -> M
  GPS : squares for channel-norms, partition broadcasts
Inverse norms via exp(-0.5*ln(normsq)) on ACT (same table set as Exp).
Host combines the tiny per-core partials into the scalar loss.
"""
import numpy as np
import ml_dtypes

import concourse.bacc as bacc
import concourse.tile as tile
from concourse import mybir
from concourse.bass_utils import run_bass_kernel_spmd

dt = mybir.dt
AF = mybir.ActivationFunctionType
ALU = mybir.AluOpType

N, C, H, W = 4, 256, 64, 64
HW = H * W                      # 4096
PH = HW // 2                    # 2048  p-half per core
NPT = PH // 128                 # 16    p-tiles
QB = 1024                       # q big-chunk
NQB = HW // QB                  # 4
NCLS = 5
TEMP_INV = 20.0
LN20 = float(np.log(TEMP_INV))
EPS_DEN = 1e-6
EPS_NSQ = 1e-24                 # clamp on ||x||^2  (=(1e-12)^2)

_CACHED_NC = None

_TABLES_PATCHED = False


def _patch_activation_tables():
    """Keep Exp/Ln only in natural_log_exp_and_others so the compiler
    loads one ACT table set instead of thrashing between exp/ln sets."""
    global _TABLES_PATCHED
    if _TABLES_PATCHED:
        return
    _TABLES_PATCHED = True
    import concourse.hw_specs as hw_specs
    import concourse.bacc as _bacc
    orig = hw_specs.get_activation_tables

    def patched(arch):
        tabs = {k: set(v) for k, v in orig(arch).items()}
        exp, ln = AF.Exp, AF.Ln
        for name, fns in tabs.items():
            if name != "natural_log_exp_and_others":
                fns.discard(exp)
                fns.discard(ln)
        return tabs

    hw_specs.get_activation_tables = patched
    if getattr(_bacc, "get_activation_tables", None) is orig:
        _bacc.get_activation_tables = patched


def build_nc():
    _patch_activation_tables()
    nc = bacc.Bacc("TRN2", target_bir_lowering=False, debug=False, num_devices=8)

    rgb_in = nc.dram_tensor("rgb_half", [2, 128, PH], dt.float32, kind="ExternalInput").ap()
    ir_in = nc.dram_tensor("ir_full", [2, 128, HW], dt.float32, kind="ExternalInput").ap()
    im_in = nc.dram_tensor("im_bcast", [128, HW], dt.float32, kind="ExternalInput").ap()
    rm_in = nc.dram_tensor("rm_cols", [128, NPT], dt.float32, kind="ExternalInput").ap()
    oh_in = nc.dram_tensor("oh_lhsT", [128, NPT * 6], dt.float32, kind="ExternalInput").ap()

    S_out = nc.dram_tensor("S_out", [NPT, 128], dt.float32, kind="ExternalOutput").ap()
    M_out = nc.dram_tensor("M_out", [NPT, 128], dt.float32, kind="ExternalOutput").ap()
    C_out = nc.dram_tensor("C_out", [6, HW], dt.float32, kind="ExternalOutput").ap()

    with tile.TileContext(nc) as tc:
        with tc.tile_pool(name="big", bufs=1) as big, \
             tc.tile_pool(name="ld", bufs=3) as ld, \
             tc.tile_pool(name="scr", bufs=3) as scr, \
             tc.tile_pool(name="rows", bufs=2) as rows, \
             tc.tile_pool(name="epool", bufs=4) as epool, \
             tc.tile_pool(name="dump", bufs=2) as dump, \
             tc.tile_pool(name="psL", bufs=2, space="PSUM") as psL, \
             tc.tile_pool(name="psC", bufs=2, space="PSUM") as psCp, \
             tc.tile_pool(name="psN", bufs=2, space="PSUM") as psN:

            # ---------------- small loads ----------------
            im_b = big.tile([128, HW], dt.float32)
            nc.sync.dma_start(im_b[:], im_in)
            rm_c = big.tile([128, NPT], dt.float32)
            nc.sync.dma_start(rm_c[:], rm_in)
            oh_f = big.tile([128, NPT * 6], dt.float32)
            nc.sync.dma_start(oh_f[:], oh_in)
            oh_b = big.tile([128, NPT * 6], dt.float32r)
            nc.vector.tensor_copy(oh_b[:], oh_f[:])

            ones_f = big.tile([128, 1], dt.float32)
            nc.vector.memset(ones_f[:], 1.0)
            ones_r = big.tile([128, 1], dt.float32r)
            nc.vector.tensor_copy(ones_r[:], ones_f[:])
            ln20_t = big.tile([128, 1], dt.float32)
            nc.vector.memset(ln20_t[:], LN20)

            # persistent matmul operands, one tile per 512-seg: [c0|c1] x 512
            rgb_segs = [big.tile([128, 2 * 512], dt.float32r, tag=f"rgbs{s}",
                                 name=f"rgbs{s}") for s in range(PH // 512)]
            ir_segs = [big.tile([128, 2 * 512], dt.float32r, tag=f"irs{s}",
                                name=f"irs{s}") for s in range(HW // 512)]
            ni_b = big.tile([128, HW], dt.float32)
            rn_b = big.tile([128, PH], dt.float32)

            def inv_norm_row(pn, with_ln20):
                """psum [1,512] normsq -> [1,512] row of scale/sqrt(nsq).

                rB = max(nsq, eps); row = Exp(-0.5*Ln(rB) + ln(scale))
                """
                rB = rows.tile([1, 512], dt.float32, tag="rB")
                nc.vector.tensor_scalar_max(rB[:], pn[:], EPS_NSQ)
                rC = rows.tile([1, 512], dt.float32, tag="rC")
                nc.scalar.activation(rC[:], rB[:], AF.Ln)
                rD = rows.tile([1, 512], dt.float32, tag="rD")
                bias = ln20_t[:1, :] if with_ln20 else 0.0
                nc.scalar.activation(rD[:], rC[:], AF.Exp,
                                     scale=-0.5, bias=bias)
                return rD

            # ---------------- rgb prologue (4 segs of 512) ----------------
            for s in range(PH // 512):
                seg = ld.tile([128, 2 * 512], dt.float32, tag="seg")
                nc.sync.dma_start(seg[:].rearrange("p (c m) -> p c m", c=2),
                                  rgb_in[:, :, s * 512:(s + 1) * 512].rearrange("c p m -> p c m"))
                pn = psN.tile([1, 512], dt.float32, tag="pn")
                for c in range(2):
                    sq = scr.tile([128, 512], dt.float32r, tag="sq")
                    nc.gpsimd.tensor_mul(sq[:], seg[:, c * 512:(c + 1) * 512],
                                         seg[:, c * 512:(c + 1) * 512])
                    nc.tensor.matmul(pn[:], ones_r[:], sq[:], start=(c == 0), stop=(c == 1))
                rD = inv_norm_row(pn, True)     # 20/||rgb_p||
                nc.gpsimd.partition_broadcast(rn_b[:, s * 512:(s + 1) * 512], rD[:1, :])
                for c in range(2):
                    nc.vector.tensor_mul(rgb_segs[s][:, c * 512:(c + 1) * 512],
                                         seg[:, c * 512:(c + 1) * 512],
                                         rn_b[:, s * 512:(s + 1) * 512])

            # ---------------- ir prologue (8 segs of 512) ----------------
            for s in range(HW // 512):
                seg = ld.tile([128, 2 * 512], dt.float32, tag="seg")
                nc.sync.dma_start(seg[:].rearrange("p (c m) -> p c m", c=2),
                                  ir_in[:, :, s * 512:(s + 1) * 512].rearrange("c p m -> p c m"))
                pn = psN.tile([1, 512], dt.float32, tag="pn")
                for c in range(2):
                    sq = scr.tile([128, 512], dt.float32r, tag="sq")
                    nc.gpsimd.tensor_mul(sq[:], seg[:, c * 512:(c + 1) * 512],
                                         seg[:, c * 512:(c + 1) * 512])
                    nc.tensor.matmul(pn[:], ones_r[:], sq[:], start=(c == 0), stop=(c == 1))
                rD = inv_norm_row(pn, False)    # 1/||ir_q||
                nc.gpsimd.partition_broadcast(ni_b[:, s * 512:(s + 1) * 512], rD[:1, :])
                for c in range(2):
                    nc.vector.tensor_mul(ir_segs[s][:, c * 512:(c + 1) * 512],
                                         seg[:, c * 512:(c + 1) * 512],
                                         ni_b[:, s * 512:(s + 1) * 512])

            # ---------------- main loop ----------------
            S_stat = big.tile([128, NPT * NQB], dt.float32)
            M_stat = big.tile([128, NPT * NQB], dt.float32)
            C_sb = big.tile([6, HW], dt.float32)

            for qb in range(NQB):
                psCa = psCp.tile([6, 512], dt.float32, tag="psC")
                psCb = psCp.tile([6, 512], dt.float32, tag="psC")
                for pt in range(NPT):
                    rs = rgb_segs[pt // 4]
                    po = (pt % 4) * 128
                    pl = psL.tile([128, QB], dt.float32, tag="pl")
                    for half in range(2):
                        irs = ir_segs[qb * 2 + half]
                        for c in range(2):
                            nc.tensor.matmul(
                                pl[:, half * 512:(half + 1) * 512],
                                rs[:, c * 512 + po: c * 512 + po + 128],
                                irs[:, c * 512:(c + 1) * 512],
                                start=(c == 0), stop=(c == 1))
                    e_t = epool.tile([128, QB], dt.float32r, tag="e")
                    nc.scalar.activation(e_t[:], pl[:], AF.Exp,
                                         accum_out=S_stat[:, pt * NQB + qb:
                                                          pt * NQB + qb + 1])
                    num_s = dump.tile([128, QB], dt.float32, tag="num")
                    nc.vector.scalar_tensor_tensor(
                        out=num_s[:],
                        in0=im_b[:, qb * QB:(qb + 1) * QB],
                        scalar=rm_c[:, pt:pt + 1],
                        in1=e_t[:].bitcast(dt.float32),
                        op0=ALU.is_equal, op1=ALU.mult,
                        accum_out=M_stat[:, pt * NQB + qb: pt * NQB + qb + 1])
                    for half, psC in ((0, psCa), (1, psCb)):
                        nc.tensor.matmul(psC[:], oh_b[:, pt * 6:(pt + 1) * 6],
                                         e_t[:, half * 512:(half + 1) * 512],
                                         start=(pt == 0), stop=(pt == NPT - 1))
                for half, psC in ((0, psCa), (1, psCb)):
                    nc.vector.tensor_copy(
                        C_sb[:, qb * QB + half * 512: qb * QB + (half + 1) * 512],
                        psC[:])

            # ---------------- epilogue ----------------
            S_red = big.tile([128, NPT], dt.float32)
            nc.vector.reduce_sum(S_red[:],
                                 S_stat[:].rearrange("p (pt q) -> p pt q", q=NQB),
                                 axis=mybir.AxisListType.X)
            M_red = big.tile([128, NPT], dt.float32)
            nc.vector.reduce_sum(M_red[:],
                                 M_stat[:].rearrange("p (pt q) -> p pt q", q=NQB),
                                 axis=mybir.AxisListType.X)
            nc.sync.dma_start(S_out.rearrange("pt p -> p pt"), S_red[:])
            nc.sync.dma_start(M_out.rearrange("pt p -> p pt"), M_red[:])
            nc.sync.dma_start(C_out, C_sb[:])

    nc.compile()
    return nc


def _get_nc():
    global _CACHED_NC
    if _CACHED_NC is None:
        _CACHED_NC = build_nc()
    return _CACHED_NC


def _build_in_maps(np_inputs):
    rgb_map = np.asarray(np_inputs["rgb_map"], dtype=np.float32).reshape(N, C, HW)
    ir_map = np.asarray(np_inputs["ir_map"], dtype=np.float32).reshape(N, C, HW)
    rm = np.asarray(np_inputs["rgb_mask"]).reshape(N, HW)
    im = np.asarray(np_inputs["ir_mask"]).reshape(N, HW)
    rm_f = rm.astype(np.float32)
    im_f = im.astype(np.float32)

    in_maps = []
    for core in range(8):
        n, h = core // 2, core % 2
        psl = slice(h * PH, (h + 1) * PH)
        rgb_half = np.ascontiguousarray(rgb_map[n, :, psl].reshape(2, 128, PH))
        ir_full = np.ascontiguousarray(ir_map[n].reshape(2, 128, HW))
        im_bc = np.broadcast_to(im_f[n], (128, HW)).copy()
        rm_half = rm_f[n, psl]
        rm_cols = np.ascontiguousarray(rm_half.reshape(NPT, 128).T)
        oh = np.empty((NPT, 128, 6), dtype=np.float32)
        oh[:, :, 0] = 1.0
        rm_tiles = rm_half.reshape(NPT, 128)
        for k in range(NCLS):
            oh[:, :, 1 + k] = (rm_tiles == k)
        oh_lhsT = np.ascontiguousarray(
            oh.transpose(1, 0, 2).reshape(128, NPT * 6))
        in_maps.append({
            "rgb_half": rgb_half,
            "ir_full": ir_full,
            "im_bcast": im_bc,
            "rm_cols": rm_cols,
            "oh_lhsT": oh_lhsT,
        })
    return in_maps


def kernel(rgb_map, ir_map, rgb_mask, ir_mask):
    np_inputs = {"rgb_map": rgb_map, "ir_map": ir_map,
                 "rgb_mask": rgb_mask, "ir_mask": ir_mask}
    in_maps = _build_in_maps(np_inputs)
    im = np.asarray(ir_mask).reshape(N, HW)
    rm = np.asarray(rgb_mask).reshape(N, HW)

    nc = _get_nc()
    res = run_bass_kernel_spmd(nc, in_maps, list(range(8)))

    # ---------------- host combine (tiny) ----------------
    entries = []
    for n in range(N):
        rA, rB = res.results[2 * n], res.results[2 * n + 1]
        S = np.concatenate([rA["S_out"].reshape(PH), rB["S_out"].reshape(PH)]).astype(np.float64)
        M = np.concatenate([rA["M_out"].reshape(PH), rB["M_out"].reshape(PH)]).astype(np.float64)
        C6 = rA["C_out"].astype(np.float64) + rB["C_out"].astype(np.float64)
        Ce = C6[0]
        imn = im[n]
        Mc = C6[1 + imn, np.arange(HW)]
        r_rgb = (M / (S + EPS_DEN)) * (rm[n] > 0)
        r_ir = (Mc / (Ce + EPS_DEN)) * (imn > 0)
        entries.append(r_rgb)
        entries.append(r_ir)
    L = np.concatenate(entries)
    nz = L != 0
    total = -np.log(L[nz]).sum() if nz.any() else 0.0
    count = max(float(nz.sum()), 1.0)
    return np.asarray(np.float32(total / count))


if __name__ == "__main__":
    import reference
    inputs = reference.setup_inputs()
    inputs = {k: np.asarray(v) for k, v in inputs.items()}
    out = kernel(**inputs)
    print("kernel:", out)

